# revision 10
# baseline (speedup 1.0000x reference)
"""Node2VecHypergraphConv distributed Trainium2 kernel (8 NeuronCores).

Algorithm (reference):
    x = emb @ conv_w.T
    e = Binv * segsum_edge(x[node_idx])          # node -> hyperedge
    n = Dinv * segsum_node(e[edge_idx]) + conv_b # hyperedge -> node
    y = lrelu(n); g = y.T @ y
    out = lrelu(g @ lin_w.T + lin_b)

Device mapping:
    Phase A (per-core edge shard): gather emb rows per incidence via
    dma_gather, scatter-sum into PSUM edge windows via one-hot S matmuls
    (deferring conv_w: e' = Binv * segsum(emb rows), then e = e' @ conv_w.T).
    AllGather e. Phase B (per-core node shard): gather e rows per incidence,
    same matmul scatter into PSUM node windows, finalize y tiles, accumulate
    Gram in PSUM, AllReduce, tiny final matmul.
"""
import sys

sys.path.insert(0, '/opt/trn_rl_repo')
import numpy as np

NCORES = 8
N_NODES = 50000
N_EDGES = 10000
C = 256
NEG = 0.01
LO_SPLIT = 32768
IPG = 1024            # indices per dma_gather instruction
NQ = 4                # SWDGE queues
SW_B = 12             # phase-B node windows resident per PSUM sweep


def _ceil(a, b):
    return -(-a // b)


def _wrap_idx(a):
    """int16 index vector -> dma_gather SBUF layout [128, L/16]."""
    L = a.shape[0]
    assert L % 16 == 0
    w = a.reshape(L // 16, 16).T.astype(np.int16)
    return np.ascontiguousarray(np.tile(w, (8, 1)))


def _cols(a, nchunks):
    """per-slot values [nchunks*128] -> [128, nchunks] column layout."""
    return np.ascontiguousarray(a.reshape(nchunks, 128).T)


def preprocess(edge_index, n_nodes=N_NODES, n_edges=N_EDGES):
    node_idx = np.asarray(edge_index[0], dtype=np.int64)
    edge_idx = np.asarray(edge_index[1], dtype=np.int64)
    nnz = node_idx.shape[0]
    E_PER = n_edges // NCORES
    N_PER = n_nodes // NCORES
    NW_A = _ceil(E_PER, 128)
    NW_B = _ceil(N_PER, 128)

    D = np.bincount(node_idx, minlength=n_nodes).astype(np.float32)
    B = np.bincount(edge_idx, minlength=n_edges).astype(np.float32)
    Dinv = np.where(D > 0, 1.0 / np.maximum(D, 1.0), 0.0).astype(np.float32)
    Binv = np.where(B > 0, 1.0 / np.maximum(B, 1.0), 0.0).astype(np.float32)

    # ---------------- phase A buckets: (core, half, window) ----------------
    core_a = edge_idx // E_PER
    eloc = edge_idx - core_a * E_PER
    win_a = eloc >> 7
    ecol = (eloc & 127).astype(np.float32)
    half = (node_idx >= LO_SPLIT).astype(np.int64)

    cnt_a = np.zeros((NCORES, 2, NW_A), dtype=np.int64)
    np.add.at(cnt_a, (core_a, half, win_a), 1)
    M_a = _ceil(np.max(cnt_a, axis=0), 128)  # [2, NW_A] chunks per (half, win)

    # slot base per (half, window) within each half's stream
    base_a = np.zeros((2, NW_A), dtype=np.int64)
    for h in range(2):
        base_a[h] = np.cumsum(np.concatenate([[0], M_a[h][:-1] * 128]))
    L_a = [int(M_a[h].sum()) * 128 for h in range(2)]      # slots per stream
    LP_a = [_ceil(max(L, 1), IPG) * IPG for L in L_a]       # padded stream len

    order = np.lexsort((win_a, half, core_a))
    so_core, so_half, so_win = core_a[order], half[order], win_a[order]
    so_node, so_ecol = node_idx[order], ecol[order]
    # rank within bucket
    bucket_key = (so_core * 2 + so_half) * NW_A + so_win
    changes = np.concatenate([[True], bucket_key[1:] != bucket_key[:-1]])
    starts = np.flatnonzero(changes)
    rank = np.arange(nnz) - np.repeat(starts, np.diff(np.concatenate([starts, [nnz]])))

    idx_a = [[None] * NCORES, [None] * NCORES]
    eid_a = [[None] * NCORES, [None] * NCORES]
    for c in range(NCORES):
        for h in range(2):
            gidx = np.zeros(LP_a[h], dtype=np.int64)
            gecol = np.full(L_a[h], -1.0, dtype=np.float32)
            sel = (so_core == c) & (so_half == h)
            slot = base_a[h][so_win[sel]] + rank[sel]
            gidx[slot] = so_node[sel] - h * LO_SPLIT
            gecol[slot] = so_ecol[sel]
            idx_a[h][c] = _wrap_idx(gidx.astype(np.int16))
            eid_a[h][c] = _cols(gecol, L_a[h] // 128) if L_a[h] else \
                np.zeros((128, 0), dtype=np.float32)

    # ---------------- phase B buckets: (core, window) ----------------
    core_b = node_idx // N_PER
    nloc = node_idx - core_b * N_PER
    win_b = nloc >> 7
    ncol = (nloc & 127).astype(np.float32)

    cnt_b = np.zeros((NCORES, NW_B), dtype=np.int64)
    np.add.at(cnt_b, (core_b, win_b), 1)
    M_b = _ceil(np.max(cnt_b, axis=0), 128)
    base_b = np.cumsum(np.concatenate([[0], M_b[:-1] * 128]))
    L_b = int(M_b.sum()) * 128
    LP_b = _ceil(max(L_b, 1), IPG) * IPG

    order = np.lexsort((win_b, core_b))
    sb_core, sb_win = core_b[order], win_b[order]
    sb_edge, sb_ncol = edge_idx[order], ncol[order]
    bucket_key = sb_core * NW_B + sb_win
    changes = np.concatenate([[True], bucket_key[1:] != bucket_key[:-1]])
    starts = np.flatnonzero(changes)
    rank = np.arange(nnz) - np.repeat(starts, np.diff(np.concatenate([starts, [nnz]])))

    idx_b = [None] * NCORES
    eid_b = [None] * NCORES
    for c in range(NCORES):
        gidx = np.zeros(LP_b, dtype=np.int64)
        gncol = np.full(L_b, -1.0, dtype=np.float32)
        sel = sb_core == c
        slot = base_b[sb_win[sel]] + rank[sel]
        gidx[slot] = sb_edge[sel]
        gncol[slot] = sb_ncol[sel]
        idx_b[c] = _wrap_idx(gidx.astype(np.int16))
        eid_b[c] = _cols(gncol, L_b // 128)

    # per-core per-window scale columns
    binv_cols = np.zeros((NCORES, 128, NW_A), dtype=np.float32)
    dinv_cols = np.zeros((NCORES, 128, NW_B), dtype=np.float32)
    mask_cols = np.zeros((NCORES, 128, NW_B), dtype=np.float32)
    for c in range(NCORES):
        bv = Binv[c * E_PER:(c + 1) * E_PER]
        bv = np.pad(bv, (0, NW_A * 128 - E_PER))
        binv_cols[c] = bv.reshape(NW_A, 128).T
        dv = Dinv[c * N_PER:(c + 1) * N_PER]
        dv = np.pad(dv, (0, NW_B * 128 - N_PER))
        dinv_cols[c] = dv.reshape(NW_B, 128).T
        mk = np.pad(np.ones(N_PER, np.float32), (0, NW_B * 128 - N_PER))
        mask_cols[c] = mk.reshape(NW_B, 128).T

    meta = dict(
        n_nodes=n_nodes, n_edges=n_edges, E_PER=E_PER, N_PER=N_PER,
        NW_A=NW_A, NW_B=NW_B,
        M_a=M_a, base_a=base_a, L_a=L_a, LP_a=LP_a,
        M_b=M_b, base_b=base_b, L_b=L_b, LP_b=LP_b,
    )
    percore = dict(
        idx_a_lo=idx_a[0], idx_a_hi=idx_a[1],
        eid_a_lo=eid_a[0], eid_a_hi=eid_a[1],
        idx_b=idx_b, eid_b=eid_b,
        binv_cols=binv_cols, dinv_cols=dinv_cols, mask_cols=mask_cols,
    )
    return meta, percore


def build_kernel(meta, debug=False):
    import concourse.bacc as bacc
    import concourse.mybir as mybir
    import concourse.tile as tile

    f32 = mybir.dt.float32
    i16 = mybir.dt.int16
    NW_A, NW_B = meta['NW_A'], meta['NW_B']
    E_PER, N_PER = meta['E_PER'], meta['N_PER']
    n_edges = meta['n_edges']
    M_a, M_b = meta['M_a'], meta['M_b']
    L_a, LP_a, L_b, LP_b = meta['L_a'], meta['LP_a'], meta['L_b'], meta['LP_b']
    nch_a = [L // 128 for L in L_a]
    nch_b = L_b // 128

    nc = bacc.Bacc('TRN2', num_devices=NCORES,
                   dynamic_dma_scratch_size=65536, num_swdge_queues=NQ)

    emb = nc.declare_dram_parameter("emb", [meta['n_nodes'], C], f32, isOutput=False)
    p_idx_lo = nc.declare_dram_parameter("idx_a_lo", [128, LP_a[0] // 16], i16, isOutput=False)
    p_idx_hi = nc.declare_dram_parameter("idx_a_hi", [128, LP_a[1] // 16], i16, isOutput=False)
    p_eid_lo = nc.declare_dram_parameter("eid_a_lo", [128, max(nch_a[0], 1)], f32, isOutput=False)
    p_eid_hi = nc.declare_dram_parameter("eid_a_hi", [128, max(nch_a[1], 1)], f32, isOutput=False)
    p_idx_b = nc.declare_dram_parameter("idx_b", [128, LP_b // 16], i16, isOutput=False)
    p_eid_b = nc.declare_dram_parameter("eid_b", [128, nch_b], f32, isOutput=False)
    p_binv = nc.declare_dram_parameter("binv_cols", [128, NW_A], f32, isOutput=False)
    p_dinv = nc.declare_dram_parameter("dinv_cols", [128, NW_B], f32, isOutput=False)
    p_mask = nc.declare_dram_parameter("mask_cols", [128, NW_B], f32, isOutput=False)
    p_wt = nc.declare_dram_parameter("wt", [128, 2, C], f32, isOutput=False)     # conv_w.T k-sliced
    p_lwt = nc.declare_dram_parameter("lwt", [128, 2, C], f32, isOutput=False)   # lin_w.T k-sliced
    p_cb = nc.declare_dram_parameter("convb_bc", [128, C], f32, isOutput=False)
    p_lb = nc.declare_dram_parameter("linb_bc", [128, C], f32, isOutput=False)
    p_iota = nc.declare_dram_parameter("iota", [128, 128], f32, isOutput=False)
    p_ident = nc.declare_dram_parameter("ident", [128, 128], f32, isOutput=False)
    out = nc.declare_dram_parameter("out", [C, C], f32, isOutput=True)
    if debug:
        dbg_e = nc.declare_dram_parameter("dbg_e", [n_edges, C], f32, isOutput=True)
        dbg_g = nc.declare_dram_parameter("dbg_g", [128, 2, C], f32, isOutput=True)
        dbg_y = nc.declare_dram_parameter("dbg_y", [NW_B * 128, C], f32, isOutput=True)

    gq = [0]

    def gather_stream(pool, idx_sb, src_ap, n_gather, tag):
        tiles = []
        for g in range(n_gather):
            t = pool.tile([128, IPG // 128, C], f32, tag=tag, name=f"g{tag}{g}")
            nc.gpsimd.dma_gather(
                t[:], src_ap, idx_sb[:, g * (IPG // 16):(g + 1) * (IPG // 16)],
                IPG, IPG, C, queue_num=gq[0] % NQ)
            gq[0] += 1
            tiles.append(t)
        return tiles

    with tile.TileContext(nc) as tc:
        with (
            tc.tile_pool(name="dram", bufs=1, space="DRAM") as dram,
            tc.tile_pool(name="const", bufs=1) as constp,
            tc.tile_pool(name="idx", bufs=1) as idxp,
        ):
            agin = dram.tile([E_PER, C], f32)
            efull = dram.tile([n_edges, C], f32, addr_space="Shared")
            arin = dram.tile([128, 2, C], f32)
            gfull = dram.tile([128, 2, C], f32, addr_space="Shared")

            iota = constp.tile([128, 128], f32)
            ident = constp.tile([128, 128], f32)
            wt = constp.tile([128, 2, C], f32)
            lwt = constp.tile([128, 2, C], f32)
            cb = constp.tile([128, C], f32)
            lb = constp.tile([128, C], f32)
            binv = constp.tile([128, NW_A], f32)
            dinv = constp.tile([128, NW_B], f32)
            mask = constp.tile([128, NW_B], f32)
            for dst, src in ((iota, p_iota), (ident, p_ident), (wt, p_wt),
                             (lwt, p_lwt), (cb, p_cb), (lb, p_lb),
                             (binv, p_binv), (dinv, p_dinv), (mask, p_mask)):
                nc.sync.dma_start(dst[:], src[:])

            idx_lo = idxp.tile([128, LP_a[0] // 16], i16)
            idx_hi = idxp.tile([128, LP_a[1] // 16], i16)
            idx_b = idxp.tile([128, LP_b // 16], i16)
            eid_lo = idxp.tile([128, max(nch_a[0], 1)], f32)
            eid_hi = idxp.tile([128, max(nch_a[1], 1)], f32)
            eid_b = idxp.tile([128, nch_b], f32)
            nc.sync.dma_start(idx_lo[:], p_idx_lo[:])
            nc.sync.dma_start(idx_hi[:], p_idx_hi[:])
            nc.sync.dma_start(idx_b[:], p_idx_b[:])
            nc.sync.dma_start(eid_lo[:], p_eid_lo[:])
            nc.sync.dma_start(eid_hi[:], p_eid_hi[:])
            nc.sync.dma_start(eid_b[:], p_eid_b[:])

            # ======================= PHASE A =======================
            with (
                tc.tile_pool(name="glo", bufs=3) as glo_pool,
                tc.tile_pool(name="ghi", bufs=3) as ghi_pool,
                tc.tile_pool(name="sA", bufs=4) as s_pool,
                tc.tile_pool(name="psA", bufs=1, space="PSUM") as psA,
                tc.tile_pool(name="psT", bufs=2, space="PSUM") as psT,
                tc.tile_pool(name="epA", bufs=3) as ep_pool,
            ):
                lo_hi = min(LO_SPLIT, meta['n_nodes'])
                n_g_lo = LP_a[0] // IPG if L_a[0] else 0
                n_g_hi = LP_a[1] // IPG if L_a[1] else 0
                lo_tiles = gather_stream(glo_pool, idx_lo, emb[0:lo_hi, :],
                                         n_g_lo, "glo") if n_g_lo else []
                hi_tiles = gather_stream(
                    ghi_pool, idx_hi, emb[lo_hi:meta['n_nodes'], :],
                    n_g_hi, "ghi") if n_g_hi else []

                chunk_pos = [0, 0]
                for w in range(NW_A):
                    eacc = psA.tile([128, C], f32, tag="eacc", name=f"eacc{w}")
                    n_lo, n_hi = int(M_a[0][w]), int(M_a[1][w])
                    n_tot = n_lo + n_hi
                    done = 0
                    for h, n_h, tiles, eids in ((0, n_lo, lo_tiles, eid_lo),
                                                (1, n_hi, hi_tiles, eid_hi)):
                        for j in range(n_h):
                            cpos = chunk_pos[h]
                            chunk_pos[h] += 1
                            g, slot = divmod(cpos, IPG // 128)
                            s = s_pool.tile([128, 128], f32, tag="sA", name=f"sA{w}_{done}")
                            nc.vector.tensor_scalar(
                                s[:], iota[:], eids[:, cpos:cpos + 1], None,
                                mybir.AluOpType.is_equal)
                            nc.tensor.matmul(
                                eacc[:], s[:], tiles[g][:, slot, :],
                                start=(done == 0), stop=(done == n_tot - 1))
                            done += 1
                    # epilogue: Binv scale, transpose, conv_w.T, emit e rows
                    nrow = min(128, E_PER - w * 128)
                    ep = ep_pool.tile([128, C], f32, tag="ep", name=f"ep{w}")
                    if n_tot == 0:
                        nc.vector.memset(ep[:], 0.0)
                    else:
                        nc.vector.tensor_scalar(
                            ep[:], eacc[:], binv[:, w:w + 1], None,
                            mybir.AluOpType.mult)
                    ept = ep_pool.tile([128, 2, 128], f32, tag="ept", name=f"ept{w}")
                    for ks in range(2):
                        tp = psT.tile([128, 128], f32, tag="tp", name=f"tp{w}_{ks}")
                        nc.tensor.transpose(tp[:], ep[:, ks * 128:(ks + 1) * 128],
                                            ident[:])
                        nc.vector.tensor_copy(ept[:, ks, :], tp[:])
                    epm = psT.tile([128, C], f32, tag="epm", name=f"epm{w}")
                    for ks in range(2):
                        nc.tensor.matmul(epm[:], ept[:, ks, :], wt[:, ks, :],
                                         start=(ks == 0), stop=(ks == 1))
                    esb = ep_pool.tile([128, C], f32, tag="esb", name=f"esb{w}")
                    nc.vector.tensor_copy(esb[:], epm[:])
                    nc.sync.dma_start(agin[w * 128:w * 128 + nrow, :],
                                      esb[:nrow, :])

            nc.gpsimd.collective_compute(
                "AllGather", mybir.AluOpType.bypass,
                replica_groups=[list(range(NCORES))],
                ins=[agin[:]], outs=[efull[:]])
            if debug:
                nc.sync.dma_start(dbg_e[:], efull[:])

            # ======================= PHASE B =======================
            with (
                tc.tile_pool(name="gb", bufs=4) as gb_pool,
                tc.tile_pool(name="sB", bufs=4) as sB_pool,
                tc.tile_pool(name="psB", bufs=1, space="PSUM") as psB,
                tc.tile_pool(name="psG", bufs=1, space="PSUM") as psG,
                tc.tile_pool(name="yB", bufs=3) as y_pool,
                tc.tile_pool(name="fin", bufs=1) as fin_pool,
            ):
                b_tiles = gather_stream(gb_pool, idx_b, efull[:],
                                        LP_b // IPG, "gb")
                g_ps = [psG.tile([128, C], f32, tag=f"g{hh}", name=f"g_ps{hh}")
                        for hh in range(2)]

                cpos = 0
                for w in range(NW_B):
                    nacc = psB.tile([128, C], f32, tag="nacc", name=f"nacc{w}")
                    n_w = int(M_b[w])
                    for j in range(n_w):
                        g, slot = divmod(cpos, IPG // 128)
                        cpos += 1
                        s = sB_pool.tile([128, 128], f32, tag="sB", name=f"sB{w}_{j}")
                        nc.vector.tensor_scalar(
                            s[:], iota[:], eid_b[:, cpos - 1:cpos], None,
                            mybir.AluOpType.is_equal)
                        nc.tensor.matmul(nacc[:], s[:], b_tiles[g][:, slot, :],
                                         start=(j == 0), stop=(j == n_w - 1))
                    y = y_pool.tile([128, C], f32, tag="y", name=f"y{w}")
                    yt = y_pool.tile([128, C], f32, tag="yt", name=f"yt{w}")
                    if n_w == 0:
                        nc.vector.memset(y[:], 0.0)
                    else:
                        nc.vector.tensor_scalar(
                            y[:], nacc[:], dinv[:, w:w + 1], None,
                            mybir.AluOpType.mult)
                    nc.vector.tensor_tensor(y[:], y[:], cb[:],
                                            mybir.AluOpType.add)
                    nc.vector.tensor_scalar(yt[:], y[:], NEG, None,
                                            mybir.AluOpType.mult)
                    nc.vector.tensor_tensor(y[:], y[:], yt[:],
                                            mybir.AluOpType.max)
                    if (w + 1) * 128 > N_PER:
                        nc.vector.tensor_scalar(
                            y[:], y[:], mask[:, w:w + 1], None,
                            mybir.AluOpType.mult)
                    if debug:
                        nc.sync.dma_start(dbg_y[w * 128:(w + 1) * 128, :], y[:])
                    for hh in range(2):
                        nc.tensor.matmul(
                            g_ps[hh][:], y[:, hh * 128:(hh + 1) * 128], y[:],
                            start=(w == 0), stop=(w == NW_B - 1))

                gsb = fin_pool.tile([128, 2, C], f32)
                for hh in range(2):
                    nc.vector.tensor_copy(gsb[:, hh, :], g_ps[hh][:])
                nc.sync.dma_start(arin[:], gsb[:])
                nc.gpsimd.collective_compute(
                    "AllReduce", mybir.AluOpType.add,
                    replica_groups=[list(range(NCORES))],
                    ins=[arin[:]], outs=[gfull[:]])

                if debug:
                    nc.sync.dma_start(dbg_g[:], gfull[:])
                gk = fin_pool.tile([128, 2, C], f32)
                nc.sync.dma_start(gk[:], gfull[:])
                osb = fin_pool.tile([128, 2, C], f32)
                for ih in range(2):
                    op = psB.tile([128, C], f32, tag="nacc", name=f"ops{ih}")
                    for ks in range(2):
                        nc.tensor.matmul(
                            op[:], gk[:, ks, ih * 128:(ih + 1) * 128],
                            lwt[:, ks, :], start=(ks == 0), stop=(ks == 1))
                    t = fin_pool.tile([128, C], f32, tag=f"fin{ih}")
                    nc.vector.tensor_tensor(t[:], op[:], lb[:],
                                            mybir.AluOpType.add)
                    u = fin_pool.tile([128, C], f32, tag=f"finu{ih}")
                    nc.vector.tensor_scalar(u[:], t[:], NEG, None,
                                            mybir.AluOpType.mult)
                    nc.vector.tensor_tensor(osb[:, ih, :], t[:], u[:],
                                            mybir.AluOpType.max)
                nc.sync.dma_start(out.rearrange("(h p) c -> p h c", h=2), osb[:])

    nc.compile()
    return nc


def make_in_maps(inputs, meta, percore):
    emb = np.ascontiguousarray(np.asarray(inputs['emb'], dtype=np.float32))
    conv_w = np.asarray(inputs['conv_w'], dtype=np.float32)
    conv_b = np.asarray(inputs['conv_b'], dtype=np.float32)
    lin_w = np.asarray(inputs['lin_w'], dtype=np.float32)
    lin_b = np.asarray(inputs['lin_b'], dtype=np.float32)

    wt = np.ascontiguousarray(
        conv_w.T.reshape(2, 128, C).transpose(1, 0, 2)).astype(np.float32)
    lwt = np.ascontiguousarray(
        lin_w.T.reshape(2, 128, C).transpose(1, 0, 2)).astype(np.float32)
    cb = np.ascontiguousarray(np.broadcast_to(conv_b, (128, C))).astype(np.float32)
    lb = np.ascontiguousarray(np.broadcast_to(lin_b, (128, C))).astype(np.float32)
    iota = np.ascontiguousarray(
        np.broadcast_to(np.arange(128, dtype=np.float32), (128, 128)))
    ident = np.eye(128, dtype=np.float32)

    in_maps = []
    for c in range(NCORES):
        in_maps.append(dict(
            emb=emb,
            idx_a_lo=percore['idx_a_lo'][c], idx_a_hi=percore['idx_a_hi'][c],
            eid_a_lo=percore['eid_a_lo'][c] if percore['eid_a_lo'][c].shape[1]
            else np.zeros((128, 1), np.float32),
            eid_a_hi=percore['eid_a_hi'][c] if percore['eid_a_hi'][c].shape[1]
            else np.zeros((128, 1), np.float32),
            idx_b=percore['idx_b'][c], eid_b=percore['eid_b'][c],
            binv_cols=percore['binv_cols'][c],
            dinv_cols=percore['dinv_cols'][c],
            mask_cols=percore['mask_cols'][c],
            wt=wt, lwt=lwt, convb_bc=cb, linb_bc=lb, iota=iota, ident=ident,
        ))
    return in_maps


def run(inputs, n_nodes=N_NODES, n_edges=N_EDGES, trace=False, debug=False):
    from concourse.bass_utils import run_bass_kernel_spmd
    meta, percore = preprocess(inputs['edge_index'], n_nodes, n_edges)
    nc = build_kernel(meta, debug=debug)
    in_maps = make_in_maps(inputs, meta, percore)
    res = run_bass_kernel_spmd(nc, in_maps, core_ids=list(range(NCORES)),
                               trace=trace)
    return res


def kernel(**inputs):
    res = run(inputs)
    return np.asarray(res.results[0]['out'], dtype=np.float32)


# revision 11
# speedup vs baseline: 1.3299x; 1.3299x over previous
"""Node2VecHypergraphConv distributed Trainium2 kernel (8 NeuronCores).

Algorithm (reference):
    x = emb @ conv_w.T
    e = Binv * segsum_edge(x[node_idx])          # node -> hyperedge
    n = Dinv * segsum_node(e[edge_idx]) + conv_b # hyperedge -> node
    y = lrelu(n); g = y.T @ y
    out = lrelu(g @ lin_w.T + lin_b)

Device mapping:
    Phase A (per-core edge shard): gather emb rows per incidence via
    dma_gather, scatter-sum into PSUM edge windows via one-hot S matmuls
    (deferring conv_w: e' = Binv * segsum(emb rows), then e = e' @ conv_w.T).
    AllGather e. Phase B (per-core node shard): gather e rows per incidence,
    same matmul scatter into PSUM node windows, finalize y tiles, accumulate
    Gram in PSUM, AllReduce, tiny final matmul.
"""
import sys

sys.path.insert(0, '/opt/trn_rl_repo')
import numpy as np

NCORES = 8
N_NODES = 50000
N_EDGES = 10000
C = 256
NEG = 0.01
LO_SPLIT = 32768
IPG = 1024            # indices per dma_gather instruction
NQ = 4                # SWDGE queues
SW_B = 12             # phase-B node windows resident per PSUM sweep


def _ceil(a, b):
    return -(-a // b)


def _wrap_idx(a):
    """int16 index vector -> dma_gather SBUF layout [128, L/16]."""
    L = a.shape[0]
    assert L % 16 == 0
    w = a.reshape(L // 16, 16).T.astype(np.int16)
    return np.ascontiguousarray(np.tile(w, (8, 1)))


def _s_bytes(ecol, nchunks):
    """per-slot one-hot cols [nchunks*128] (-1=pad) -> int8 [128, nchunks*128].

    S_in[p, c*128 + j] = 1 iff ecol[c*128+p] == j; chunk count padded to a
    multiple of IPG//128 so DMA groups align with gather groups."""
    ncp = _ceil(max(nchunks, 1), IPG // 128) * (IPG // 128)
    m = ecol.reshape(nchunks, 128)
    oh = (m[:, :, None] == np.arange(128, dtype=np.float32)[None, None, :])
    out = np.zeros((128, ncp * 128), dtype=np.int8)
    out[:, :nchunks * 128] = oh.transpose(1, 0, 2).reshape(128, nchunks * 128)
    return out


def preprocess(edge_index, n_nodes=N_NODES, n_edges=N_EDGES):
    node_idx = np.asarray(edge_index[0], dtype=np.int64)
    edge_idx = np.asarray(edge_index[1], dtype=np.int64)
    nnz = node_idx.shape[0]
    E_PER = n_edges // NCORES
    N_PER = n_nodes // NCORES
    NW_A = _ceil(E_PER, 128)
    NW_B = _ceil(N_PER, 128)

    D = np.bincount(node_idx, minlength=n_nodes).astype(np.float32)
    B = np.bincount(edge_idx, minlength=n_edges).astype(np.float32)
    Dinv = np.where(D > 0, 1.0 / np.maximum(D, 1.0), 0.0).astype(np.float32)
    Binv = np.where(B > 0, 1.0 / np.maximum(B, 1.0), 0.0).astype(np.float32)

    # ---------------- phase A buckets: (core, half, window) ----------------
    core_a = edge_idx // E_PER
    eloc = edge_idx - core_a * E_PER
    win_a = eloc >> 7
    ecol = (eloc & 127).astype(np.float32)
    half = (node_idx >= LO_SPLIT).astype(np.int64)

    cnt_a = np.zeros((NCORES, 2, NW_A), dtype=np.int64)
    np.add.at(cnt_a, (core_a, half, win_a), 1)
    M_a = _ceil(np.max(cnt_a, axis=0), 128)  # [2, NW_A] chunks per (half, win)

    # slot base per (half, window) within each half's stream
    base_a = np.zeros((2, NW_A), dtype=np.int64)
    for h in range(2):
        base_a[h] = np.cumsum(np.concatenate([[0], M_a[h][:-1] * 128]))
    L_a = [int(M_a[h].sum()) * 128 for h in range(2)]      # slots per stream
    LP_a = [_ceil(max(L, 1), IPG) * IPG for L in L_a]       # padded stream len

    order = np.lexsort((win_a, half, core_a))
    so_core, so_half, so_win = core_a[order], half[order], win_a[order]
    so_node, so_ecol = node_idx[order], ecol[order]
    # rank within bucket
    bucket_key = (so_core * 2 + so_half) * NW_A + so_win
    changes = np.concatenate([[True], bucket_key[1:] != bucket_key[:-1]])
    starts = np.flatnonzero(changes)
    rank = np.arange(nnz) - np.repeat(starts, np.diff(np.concatenate([starts, [nnz]])))

    idx_a = [[None] * NCORES, [None] * NCORES]
    eid_a = [[None] * NCORES, [None] * NCORES]
    for c in range(NCORES):
        for h in range(2):
            gidx = np.zeros(LP_a[h], dtype=np.int64)
            gecol = np.full(L_a[h], -1.0, dtype=np.float32)
            sel = (so_core == c) & (so_half == h)
            slot = base_a[h][so_win[sel]] + rank[sel]
            gidx[slot] = so_node[sel] - h * LO_SPLIT
            gecol[slot] = so_ecol[sel]
            idx_a[h][c] = _wrap_idx(gidx.astype(np.int16))
            eid_a[h][c] = _s_bytes(gecol, L_a[h] // 128) if L_a[h] else \
                np.zeros((128, IPG), dtype=np.int8)

    # ---------------- phase B buckets: (core, window) ----------------
    core_b = node_idx // N_PER
    nloc = node_idx - core_b * N_PER
    win_b = nloc >> 7
    ncol = (nloc & 127).astype(np.float32)

    cnt_b = np.zeros((NCORES, NW_B), dtype=np.int64)
    np.add.at(cnt_b, (core_b, win_b), 1)
    M_b = _ceil(np.max(cnt_b, axis=0), 128)
    base_b = np.cumsum(np.concatenate([[0], M_b[:-1] * 128]))
    L_b = int(M_b.sum()) * 128
    LP_b = _ceil(max(L_b, 1), IPG) * IPG

    order = np.lexsort((win_b, core_b))
    sb_core, sb_win = core_b[order], win_b[order]
    sb_edge, sb_ncol = edge_idx[order], ncol[order]
    bucket_key = sb_core * NW_B + sb_win
    changes = np.concatenate([[True], bucket_key[1:] != bucket_key[:-1]])
    starts = np.flatnonzero(changes)
    rank = np.arange(nnz) - np.repeat(starts, np.diff(np.concatenate([starts, [nnz]])))

    idx_b = [None] * NCORES
    eid_b = [None] * NCORES
    for c in range(NCORES):
        gidx = np.zeros(LP_b, dtype=np.int64)
        gncol = np.full(L_b, -1.0, dtype=np.float32)
        sel = sb_core == c
        slot = base_b[sb_win[sel]] + rank[sel]
        gidx[slot] = sb_edge[sel]
        gncol[slot] = sb_ncol[sel]
        idx_b[c] = _wrap_idx(gidx.astype(np.int16))
        eid_b[c] = _s_bytes(gncol, L_b // 128)

    # per-core per-window scale columns
    binv_cols = np.zeros((NCORES, 128, NW_A), dtype=np.float32)
    dinv_cols = np.zeros((NCORES, 128, NW_B), dtype=np.float32)
    mask_cols = np.zeros((NCORES, 128, NW_B), dtype=np.float32)
    for c in range(NCORES):
        bv = Binv[c * E_PER:(c + 1) * E_PER]
        bv = np.pad(bv, (0, NW_A * 128 - E_PER))
        binv_cols[c] = bv.reshape(NW_A, 128).T
        dv = Dinv[c * N_PER:(c + 1) * N_PER]
        dv = np.pad(dv, (0, NW_B * 128 - N_PER))
        dinv_cols[c] = dv.reshape(NW_B, 128).T
        mk = np.pad(np.ones(N_PER, np.float32), (0, NW_B * 128 - N_PER))
        mask_cols[c] = mk.reshape(NW_B, 128).T

    meta = dict(
        n_nodes=n_nodes, n_edges=n_edges, E_PER=E_PER, N_PER=N_PER,
        NW_A=NW_A, NW_B=NW_B,
        M_a=M_a, base_a=base_a, L_a=L_a, LP_a=LP_a,
        M_b=M_b, base_b=base_b, L_b=L_b, LP_b=LP_b,
    )
    percore = dict(
        idx_a_lo=idx_a[0], idx_a_hi=idx_a[1],
        eid_a_lo=eid_a[0], eid_a_hi=eid_a[1],
        idx_b=idx_b, eid_b=eid_b,
        binv_cols=binv_cols, dinv_cols=dinv_cols, mask_cols=mask_cols,
    )
    return meta, percore


def build_kernel(meta, debug=False):
    import concourse.bacc as bacc
    import concourse.mybir as mybir
    import concourse.tile as tile

    f32 = mybir.dt.float32
    i16 = mybir.dt.int16
    i8 = mybir.dt.int8
    NW_A, NW_B = meta['NW_A'], meta['NW_B']
    E_PER, N_PER = meta['E_PER'], meta['N_PER']
    n_edges = meta['n_edges']
    M_a, M_b = meta['M_a'], meta['M_b']
    L_a, LP_a, L_b, LP_b = meta['L_a'], meta['LP_a'], meta['L_b'], meta['LP_b']
    nch_a = [L // 128 for L in L_a]
    nch_b = L_b // 128

    nc = bacc.Bacc('TRN2', num_devices=NCORES,
                   dynamic_dma_scratch_size=65536, num_swdge_queues=NQ)

    emb = nc.declare_dram_parameter("emb", [meta['n_nodes'], C], f32, isOutput=False)
    p_idx_lo = nc.declare_dram_parameter("idx_a_lo", [128, LP_a[0] // 16], i16, isOutput=False)
    p_idx_hi = nc.declare_dram_parameter("idx_a_hi", [128, LP_a[1] // 16], i16, isOutput=False)
    ncp_a = [_ceil(max(n, 1), IPG // 128) * (IPG // 128) for n in nch_a]
    ncp_b = _ceil(max(nch_b, 1), IPG // 128) * (IPG // 128)
    p_eid_lo = nc.declare_dram_parameter("eid_a_lo", [128, ncp_a[0] * 128], i8, isOutput=False)
    p_eid_hi = nc.declare_dram_parameter("eid_a_hi", [128, ncp_a[1] * 128], i8, isOutput=False)
    p_idx_b = nc.declare_dram_parameter("idx_b", [128, LP_b // 16], i16, isOutput=False)
    p_eid_b = nc.declare_dram_parameter("eid_b", [128, ncp_b * 128], i8, isOutput=False)
    p_binv = nc.declare_dram_parameter("binv_cols", [128, NW_A], f32, isOutput=False)
    p_dinv = nc.declare_dram_parameter("dinv_cols", [128, NW_B], f32, isOutput=False)
    p_mask = nc.declare_dram_parameter("mask_cols", [128, NW_B], f32, isOutput=False)
    p_wt = nc.declare_dram_parameter("wt", [128, 2, C], f32, isOutput=False)     # conv_w.T k-sliced
    p_lwt = nc.declare_dram_parameter("lwt", [128, 2, C], f32, isOutput=False)   # lin_w.T k-sliced
    p_cb = nc.declare_dram_parameter("convb_bc", [128, C], f32, isOutput=False)
    p_lb = nc.declare_dram_parameter("linb_bc", [128, C], f32, isOutput=False)
    p_iota = nc.declare_dram_parameter("iota", [128, 128], f32, isOutput=False)
    p_ident = nc.declare_dram_parameter("ident", [128, 128], f32, isOutput=False)
    out = nc.declare_dram_parameter("out", [C, C], f32, isOutput=True)
    if debug:
        dbg_e = nc.declare_dram_parameter("dbg_e", [n_edges, C], f32, isOutput=True)
        dbg_g = nc.declare_dram_parameter("dbg_g", [128, 2, C], f32, isOutput=True)
        dbg_y = nc.declare_dram_parameter("dbg_y", [NW_B * 128, C], f32, isOutput=True)

    gq = [0]

    def gather_stream(pool, spool, idx_sb, s_param, src_ap, n_gather, n_sgroups, tag):
        tiles, stiles = [], []
        GC = IPG // 128
        for g in range(n_gather):
            t = pool.tile([128, GC, C], f32, tag=tag, name=f"g{tag}{g}")
            nc.gpsimd.dma_gather(
                t[:], src_ap, idx_sb[:, g * (IPG // 16):(g + 1) * (IPG // 16)],
                IPG, IPG, C, queue_num=gq[0] % NQ)
            gq[0] += 1
            tiles.append(t)
            if g < n_sgroups:
                si = spool.tile([128, GC * 128], i8, tag=f"si{tag}", name=f"si{tag}{g}")
                nc.sync.dma_start(si[:], s_param[:, g * IPG:(g + 1) * IPG])
                sf = spool.tile([128, GC, 128], f32, tag=f"sf{tag}", name=f"sf{tag}{g}")
                nc.vector.tensor_copy(
                    sf[:], si.rearrange("p (c j) -> p c j", c=GC))
                stiles.append(sf)
        return tiles, stiles

    with tile.TileContext(nc) as tc:
        with (
            tc.tile_pool(name="dram", bufs=1, space="DRAM") as dram,
            tc.tile_pool(name="const", bufs=1) as constp,
            tc.tile_pool(name="idx", bufs=1) as idxp,
        ):
            agin = dram.tile([E_PER, C], f32)
            efull = dram.tile([n_edges, C], f32, addr_space="Shared")
            arin = dram.tile([128, 2, C], f32)
            gfull = dram.tile([128, 2, C], f32, addr_space="Shared")

            iota = constp.tile([128, 128], f32)
            ident = constp.tile([128, 128], f32)
            wt = constp.tile([128, 2, C], f32)
            lwt = constp.tile([128, 2, C], f32)
            cb = constp.tile([128, C], f32)
            lb = constp.tile([128, C], f32)
            binv = constp.tile([128, NW_A], f32)
            dinv = constp.tile([128, NW_B], f32)
            mask = constp.tile([128, NW_B], f32)
            for dst, src in ((iota, p_iota), (ident, p_ident), (wt, p_wt),
                             (lwt, p_lwt), (cb, p_cb), (lb, p_lb),
                             (binv, p_binv), (dinv, p_dinv), (mask, p_mask)):
                nc.sync.dma_start(dst[:], src[:])

            idx_lo = idxp.tile([128, LP_a[0] // 16], i16)
            idx_hi = idxp.tile([128, LP_a[1] // 16], i16)
            idx_b = idxp.tile([128, LP_b // 16], i16)
            nc.sync.dma_start(idx_lo[:], p_idx_lo[:])
            nc.sync.dma_start(idx_hi[:], p_idx_hi[:])
            nc.sync.dma_start(idx_b[:], p_idx_b[:])

            # ======================= PHASE A =======================
            with (
                tc.tile_pool(name="glo", bufs=3) as glo_pool,
                tc.tile_pool(name="ghi", bufs=3) as ghi_pool,
                tc.tile_pool(name="sA", bufs=4) as s_pool,
                tc.tile_pool(name="psA", bufs=1, space="PSUM") as psA,
                tc.tile_pool(name="psT", bufs=2, space="PSUM") as psT,
                tc.tile_pool(name="epA", bufs=3) as ep_pool,
            ):
                lo_hi = min(LO_SPLIT, meta['n_nodes'])
                n_g_lo = LP_a[0] // IPG if L_a[0] else 0
                n_g_hi = LP_a[1] // IPG if L_a[1] else 0
                lo_tiles, lo_s = gather_stream(
                    glo_pool, s_pool, idx_lo, p_eid_lo, emb[0:lo_hi, :],
                    n_g_lo, _ceil(nch_a[0], IPG // 128), "glo") \
                    if n_g_lo else ([], [])
                hi_tiles, hi_s = gather_stream(
                    ghi_pool, s_pool, idx_hi, p_eid_hi,
                    emb[lo_hi:meta['n_nodes'], :],
                    n_g_hi, _ceil(nch_a[1], IPG // 128), "ghi") \
                    if n_g_hi else ([], [])

                chunk_pos = [0, 0]
                for w in range(NW_A):
                    eacc = psA.tile([128, C], f32, tag="eacc", name=f"eacc{w}")
                    n_lo, n_hi = int(M_a[0][w]), int(M_a[1][w])
                    n_tot = n_lo + n_hi
                    done = 0
                    for h, n_h, tiles, stiles in ((0, n_lo, lo_tiles, lo_s),
                                                  (1, n_hi, hi_tiles, hi_s)):
                        for j in range(n_h):
                            cpos = chunk_pos[h]
                            chunk_pos[h] += 1
                            g, slot = divmod(cpos, IPG // 128)
                            nc.tensor.matmul(
                                eacc[:], stiles[g][:, slot, :],
                                tiles[g][:, slot, :],
                                start=(done == 0), stop=(done == n_tot - 1))
                            done += 1
                    # epilogue: Binv scale, transpose, conv_w.T, emit e rows
                    nrow = min(128, E_PER - w * 128)
                    ep = ep_pool.tile([128, C], f32, tag="ep", name=f"ep{w}")
                    if n_tot == 0:
                        nc.vector.memset(ep[:], 0.0)
                    else:
                        nc.vector.tensor_scalar(
                            ep[:], eacc[:], binv[:, w:w + 1], None,
                            mybir.AluOpType.mult)
                    ept = ep_pool.tile([128, 2, 128], f32, tag="ept", name=f"ept{w}")
                    for ks in range(2):
                        tp = psT.tile([128, 128], f32, tag="tp", name=f"tp{w}_{ks}")
                        nc.tensor.transpose(tp[:], ep[:, ks * 128:(ks + 1) * 128],
                                            ident[:])
                        nc.vector.tensor_copy(ept[:, ks, :], tp[:])
                    epm = psT.tile([128, C], f32, tag="epm", name=f"epm{w}")
                    for ks in range(2):
                        nc.tensor.matmul(epm[:], ept[:, ks, :], wt[:, ks, :],
                                         start=(ks == 0), stop=(ks == 1))
                    esb = ep_pool.tile([128, C], f32, tag="esb", name=f"esb{w}")
                    nc.vector.tensor_copy(esb[:], epm[:])
                    nc.sync.dma_start(agin[w * 128:w * 128 + nrow, :],
                                      esb[:nrow, :])

            nc.gpsimd.collective_compute(
                "AllGather", mybir.AluOpType.bypass,
                replica_groups=[list(range(NCORES))],
                ins=[agin[:]], outs=[efull[:]])
            if debug:
                nc.sync.dma_start(dbg_e[:], efull[:])

            # ======================= PHASE B =======================
            with (
                tc.tile_pool(name="gb", bufs=4) as gb_pool,
                tc.tile_pool(name="sB", bufs=4) as sB_pool,
                tc.tile_pool(name="psB", bufs=1, space="PSUM") as psB,
                tc.tile_pool(name="psG", bufs=1, space="PSUM") as psG,
                tc.tile_pool(name="yB", bufs=3) as y_pool,
                tc.tile_pool(name="fin", bufs=1) as fin_pool,
            ):
                b_tiles, b_s = gather_stream(
                    gb_pool, sB_pool, idx_b, p_eid_b, efull[:],
                    LP_b // IPG, _ceil(nch_b, IPG // 128), "gb")
                g_ps = [psG.tile([128, C], f32, tag=f"g{hh}", name=f"g_ps{hh}")
                        for hh in range(2)]

                cpos = 0
                for w in range(NW_B):
                    nacc = psB.tile([128, C], f32, tag="nacc", name=f"nacc{w}")
                    n_w = int(M_b[w])
                    for j in range(n_w):
                        g, slot = divmod(cpos, IPG // 128)
                        cpos += 1
                        nc.tensor.matmul(nacc[:], b_s[g][:, slot, :],
                                         b_tiles[g][:, slot, :],
                                         start=(j == 0), stop=(j == n_w - 1))
                    y = y_pool.tile([128, C], f32, tag="y", name=f"y{w}")
                    yt = y_pool.tile([128, C], f32, tag="yt", name=f"yt{w}")
                    if n_w == 0:
                        nc.vector.memset(y[:], 0.0)
                    else:
                        nc.vector.tensor_scalar(
                            y[:], nacc[:], dinv[:, w:w + 1], None,
                            mybir.AluOpType.mult)
                    nc.vector.tensor_tensor(y[:], y[:], cb[:],
                                            mybir.AluOpType.add)
                    nc.vector.tensor_scalar(yt[:], y[:], NEG, None,
                                            mybir.AluOpType.mult)
                    nc.vector.tensor_tensor(y[:], y[:], yt[:],
                                            mybir.AluOpType.max)
                    if (w + 1) * 128 > N_PER:
                        nc.vector.tensor_scalar(
                            y[:], y[:], mask[:, w:w + 1], None,
                            mybir.AluOpType.mult)
                    if debug:
                        nc.sync.dma_start(dbg_y[w * 128:(w + 1) * 128, :], y[:])
                    for hh in range(2):
                        nc.tensor.matmul(
                            g_ps[hh][:], y[:, hh * 128:(hh + 1) * 128], y[:],
                            start=(w == 0), stop=(w == NW_B - 1))

                gsb = fin_pool.tile([128, 2, C], f32)
                for hh in range(2):
                    nc.vector.tensor_copy(gsb[:, hh, :], g_ps[hh][:])
                nc.sync.dma_start(arin[:], gsb[:])
                nc.gpsimd.collective_compute(
                    "AllReduce", mybir.AluOpType.add,
                    replica_groups=[list(range(NCORES))],
                    ins=[arin[:]], outs=[gfull[:]])

                if debug:
                    nc.sync.dma_start(dbg_g[:], gfull[:])
                gk = fin_pool.tile([128, 2, C], f32)
                nc.sync.dma_start(gk[:], gfull[:])
                osb = fin_pool.tile([128, 2, C], f32)
                for ih in range(2):
                    op = psB.tile([128, C], f32, tag="nacc", name=f"ops{ih}")
                    for ks in range(2):
                        nc.tensor.matmul(
                            op[:], gk[:, ks, ih * 128:(ih + 1) * 128],
                            lwt[:, ks, :], start=(ks == 0), stop=(ks == 1))
                    t = fin_pool.tile([128, C], f32, tag=f"fin{ih}")
                    nc.vector.tensor_tensor(t[:], op[:], lb[:],
                                            mybir.AluOpType.add)
                    u = fin_pool.tile([128, C], f32, tag=f"finu{ih}")
                    nc.vector.tensor_scalar(u[:], t[:], NEG, None,
                                            mybir.AluOpType.mult)
                    nc.vector.tensor_tensor(osb[:, ih, :], t[:], u[:],
                                            mybir.AluOpType.max)
                nc.sync.dma_start(out.rearrange("(h p) c -> p h c", h=2), osb[:])

    nc.compile()
    return nc


def make_in_maps(inputs, meta, percore):
    emb = np.ascontiguousarray(np.asarray(inputs['emb'], dtype=np.float32))
    conv_w = np.asarray(inputs['conv_w'], dtype=np.float32)
    conv_b = np.asarray(inputs['conv_b'], dtype=np.float32)
    lin_w = np.asarray(inputs['lin_w'], dtype=np.float32)
    lin_b = np.asarray(inputs['lin_b'], dtype=np.float32)

    wt = np.ascontiguousarray(
        conv_w.T.reshape(2, 128, C).transpose(1, 0, 2)).astype(np.float32)
    lwt = np.ascontiguousarray(
        lin_w.T.reshape(2, 128, C).transpose(1, 0, 2)).astype(np.float32)
    cb = np.ascontiguousarray(np.broadcast_to(conv_b, (128, C))).astype(np.float32)
    lb = np.ascontiguousarray(np.broadcast_to(lin_b, (128, C))).astype(np.float32)
    iota = np.ascontiguousarray(
        np.broadcast_to(np.arange(128, dtype=np.float32), (128, 128)))
    ident = np.eye(128, dtype=np.float32)

    in_maps = []
    for c in range(NCORES):
        in_maps.append(dict(
            emb=emb,
            idx_a_lo=percore['idx_a_lo'][c], idx_a_hi=percore['idx_a_hi'][c],
            eid_a_lo=percore['eid_a_lo'][c],
            eid_a_hi=percore['eid_a_hi'][c],
            idx_b=percore['idx_b'][c], eid_b=percore['eid_b'][c],
            binv_cols=percore['binv_cols'][c],
            dinv_cols=percore['dinv_cols'][c],
            mask_cols=percore['mask_cols'][c],
            wt=wt, lwt=lwt, convb_bc=cb, linb_bc=lb, iota=iota, ident=ident,
        ))
    return in_maps


def run(inputs, n_nodes=N_NODES, n_edges=N_EDGES, trace=False, debug=False):
    from concourse.bass_utils import run_bass_kernel_spmd
    meta, percore = preprocess(inputs['edge_index'], n_nodes, n_edges)
    nc = build_kernel(meta, debug=debug)
    in_maps = make_in_maps(inputs, meta, percore)
    res = run_bass_kernel_spmd(nc, in_maps, core_ids=list(range(NCORES)),
                               trace=trace)
    return res


def kernel(**inputs):
    res = run(inputs)
    return np.asarray(res.results[0]['out'], dtype=np.float32)
